# revision 38
# baseline (speedup 1.0000x reference)
"""AnomalyScores (PatchCore-style 1-NN retrieval) Trainium2 kernel.

Sharding: data-parallel over the batch dim - core i owns batch i's 784
patches; the 16384x384 coreset is replicated on every core. All compute
is core-local; no collectives.

Per-core pipeline (fp8e4m3 DoubleRow matmuls, sign-flipped so every
selection is an argMAX):
  1. PE computes v = 2E @ C^T - b2' as [112 x 1024] PSUM tiles via two
     K=256 DoubleRow matmuls per 512-col bank; the centered-coreset-norm
     bias (-b2', two fp8 residual rows) rides in the second DoubleRow's
     spare contraction rows, so PSUM needs no preload.
  2. Drain (PSUM is readable only by DVE/ACT, one PSUM operand/instr):
     D-tiles: DVE tensor_reduce max -> mvs.  E-tiles: ACT computes
     exp(s*(v-K)) with accum_out -> per-tile sum; log-sum-exp recovers
     the row max to ~0.03 (only the patch argmax consumes these, so the
     softmax bias is harmless).  Tiles interleave D/E to balance both
     engines; Pool cannot touch tensors on HW and stays idle.
  3. scores^2 = a2c - rowmax, batched [112, 7]; patch argmax via PE
     transpose of a [128, 3] (score, patch-idx, a2c) combo. No DRAM.
  4. Part A: v-row of the max patch as a [128, 128] PSUM grid (256 tiny
     DoubleRow matmuls, rhs = dynamic fp8 column of emt); argmax -> nn.
     ACT takes ln(a2c[mp] - v) of the whole grid once (lgrid); sqrt is
     never needed: d = exp(0.5*ln(d^2)), keeping every activation in
     the one exp+ln+square table set (no tail table switches).
  5. Part B: same grid shape for 2*C[nn] vs C; top-4 per partition + PE
     transposes -> [1, 512] candidate row; max8/match_replace/max8 ->
     top-9 positions; indirect_copy gathers their lgrid values.
  6. softmax: d9 = exp(0.5*l9); exp+accum -> sum; score = min(d9);
     out = (1 - exp(score)/sum) * score. One scalar DMA per core.
"""

import sys

import numpy as np
import ml_dtypes

if "/opt/trn_rl_repo" not in sys.path:
    sys.path.insert(0, "/opt/trn_rl_repo")

import concourse.bass as bass
import concourse.mybir as mybir
import concourse.tile as tile
from concourse import bacc
from concourse.bass import ds
from concourse.bass_utils import run_bass_kernel_spmd

FP8NP = ml_dtypes.float8_e4m3
F32 = mybir.dt.float32
BF = mybir.dt.bfloat16
FP8 = mybir.dt.float8e4
U32 = mybir.dt.uint32
U16 = mybir.dt.uint16

B, P, D, N = 8, 784, 384, 16384
PT = 112          # patches per M-tile (7 * 112 = 784)
MT = 7
NG = 16           # N groups of 1024 (2 PSUM banks each)
NJ = 2
C0 = 384.0
BIG = 3.0e38
SM = 0.4          # log-sum-exp sharpness for the E-path drain
KG = 218.0        # global v-offset: |v_max - KG| stays well inside exp range
ND = 58           # of the 112 (m, g) tiles, this many go to the DVE path

Alu = mybir.AluOpType
Act = mybir.ActivationFunctionType
Axis = mybir.AxisListType
PM = mybir.MatmulPerfMode
Eng = mybir.EngineType


def _build(stage=99):
    nc = _build_inner(stage)
    nc.finalize()
    return nc


def _build_inner(stage=99):
    nc = bacc.Bacc("TRN2", target_bir_lowering=False, debug=False)

    ct8a_d = nc.dram_tensor("ct8a", [128, 2 * N], FP8, kind="ExternalInput")
    ct8b_d = nc.dram_tensor("ct8b", [128, 2 * N], FP8, kind="ExternalInput")
    emt8a_d = nc.dram_tensor("emt8a", [128, 2 * P], FP8, kind="ExternalInput")
    emt8b_d = nc.dram_tensor("emt8b", [128, 2 * P], FP8, kind="ExternalInput")
    er_d = nc.dram_tensor("er", [P, D], F32, kind="ExternalInput")
    id_d = nc.dram_tensor("ident", [128, 128], F32, kind="ExternalInput")
    out_d = nc.dram_tensor("out", [1], F32, kind="ExternalOutput")

    with tile.TileContext(nc) as tc:
        with (
            tc.tile_pool(name="constp", bufs=1) as constp,
            tc.tile_pool(name="workp", bufs=2) as workp,
            tc.tile_pool(name="psump", bufs=4, space="PSUM") as psump,
        ):
            # ---------------- resident inputs ----------------
            emt8a = constp.tile([128, 2, P], FP8, name="emt8a")
            nc.sync.dma_start(
                out=emt8a, in_=emt8a_d[:, :].rearrange("p (i n) -> p i n", i=2)
            )
            emt8b = constp.tile([128, 2, P], FP8, name="emt8b")
            nc.sync.dma_start(
                out=emt8b, in_=emt8b_d[:, :].rearrange("p (i n) -> p i n", i=2)
            )
            ident = constp.tile([128, 128], F32, name="ident")
            nc.sync.dma_start(out=ident, in_=id_d[:, :])

            ct8a = constp.tile([128, 2, N], FP8, name="ct8a")
            ct8b = constp.tile([128, 2, N], FP8, name="ct8b")
            ct8a_v = ct8a_d[:, :].rearrange("p (i n) -> p i n", i=2)
            ct8b_v = ct8b_d[:, :].rearrange("p (i n) -> p i n", i=2)
            for g in range(NG):
                lo, hi = g * 1024, (g + 1) * 1024
                nc.sync.dma_start(out=ct8a[:, :, lo:hi], in_=ct8a_v[:, :, lo:hi])
                nc.sync.dma_start(out=ct8b[:, :, lo:hi], in_=ct8b_v[:, :, lo:hi])

            # PE pstate warmup on a zeroed junk tile while DMAs land.
            wj = constp.tile([128, 512], BF, name="wj")
            nc.vector.memset(wj, 0.0)
            for _ in range(12):
                wps = psump.tile([128, NJ, 512], F32, name="wps", tag="ps")
                nc.tensor.matmul(
                    wps[:, 0, :], lhsT=wj[:, 0:128], rhs=wj, start=True, stop=True
                )

            pidx = constp.tile([128, 1], U32, name="pidx")
            nc.gpsimd.iota(pidx, pattern=[[0, 1]], base=0, channel_multiplier=1)
            ones1 = constp.tile([1, 128], F32, name="ones1")
            nc.vector.memset(ones1, 1.0)

            # prezeroed tiles touched by indirect gathers / partial writes
            a2cs = constp.tile([128, 8], F32, name="a2cs")
            nc.vector.memset(a2cs, 0.0)
            scol = constp.tile([128, 8], F32, name="scol")
            nc.vector.memset(scol, -BIG)
            # part-B rhs second k-pair: plane 1 is the constant bias rhs
            # (rows 0,1 = 1.0, rest 0), plane 0 filled at tail time.
            ccol8b = constp.tile([128, 2, 1], FP8, name="ccol8b")
            nc.vector.memset(ccol8b, 0.0)
            nc.vector.memset(ccol8b[0:2, 1, 0:1], 1.0)

            pidxf = constp.tile([128, 1], F32, name="pidxf")
            nc.vector.tensor_copy(pidxf, pidx)
            iotam = constp.tile([128, 9], F32, name="iotam")
            nc.gpsimd.iota(iotam, pattern=[[1, 9]], base=0, channel_multiplier=0, allow_small_or_imprecise_dtypes=True)
            iotan = constp.tile([128, 128], F32, name="iotan")
            nc.gpsimd.iota(iotan, pattern=[[1, 128]], base=0, channel_multiplier=0, allow_small_or_imprecise_dtypes=True)
            ones128 = constp.tile([128, 1], F32, name="ones128")
            nc.vector.memset(ones128, 1.0)
            ebias = constp.tile([PT, 1], F32, name="ebias")
            nc.vector.memset(ebias, -SM * KG)
            # drain accumulators: row maxes (D) and exp-sums (E) per (m, g)
            mvs = constp.tile([PT, MT, NG], F32, name="mvs")
            nc.vector.memset(mvs, -BIG)
            esums = constp.tile([PT, MT, NG], F32, name="esums")
            nc.vector.memset(esums, 0.0)

            # Preload the exp+ln+square act table once (set 6) while DMAs
            # stream; the greedy auto-inserter would otherwise ping-pong
            # between the exp-only and ln-only sets in the tail.
            nc.scalar.add_instruction(
                mybir.InstLoadActFuncSet(
                    name=nc.get_next_instruction_name(), ins=[], outs=[],
                    act_func_set_id=6,
                )
            )

            # ---------------- a2 + C0 per M-tile (ACT; overlaps main) -----
            for m in range(MT):
                er_sb = workp.tile([PT, D], F32, name="er_sb", tag="er_sb")
                nc.sync.dma_start(out=er_sb, in_=er_d[m * PT : (m + 1) * PT, :])
                sq = workp.tile([PT, D], F32, name="sq", tag="sq")
                a2r = workp.tile([PT, 1], F32, name="a2r", tag="a2r")
                nc.scalar.activation(out=sq, in_=er_sb, func=Act.Square, accum_out=a2r)
                nc.vector.tensor_scalar_add(a2cs[0:PT, m : m + 1], a2r, C0)

            a2K = constp.tile([PT, MT], F32, name="a2K")
            nc.vector.tensor_scalar_add(a2K, a2cs[0:PT, 0:MT], -KG)

            # ---------------- main distance pass ----------------
            k = 0
            for g in range(NG):
                for m in range(MT):
                    ps = psump.tile([128, NJ, 512], F32, name="ps", tag="ps")
                    for j in range(NJ):
                        col = (g * NJ + j) * 512
                        nc.tensor.matmul(
                            ps[0:PT, j, :],
                            lhsT=emt8a[:, :, m * PT : (m + 1) * PT],
                            rhs=ct8a[:, :, col : col + 512],
                            start=True,
                            stop=False,
                            perf_mode=PM.DoubleRow,
                        )
                        nc.tensor.matmul(
                            ps[0:PT, j, :],
                            lhsT=emt8b[:, :, m * PT : (m + 1) * PT],
                            rhs=ct8b[:, :, col : col + 512],
                            start=False,
                            stop=True,
                            perf_mode=PM.DoubleRow,
                        )
                    if (k * ND) % 112 < ND:
                        nc.vector.tensor_reduce(
                            out=mvs[:, m, g : g + 1], in_=ps[0:PT],
                            axis=Axis.XY, op=Alu.max,
                        )
                    else:
                        scrE = workp.tile(
                            [PT, NJ, 512], BF, name="scrE", tag="scrE", bufs=3
                        )
                        nc.scalar.activation(
                            out=scrE, in_=ps[0:PT], func=Act.Exp,
                            scale=SM, bias=ebias,
                            accum_out=esums[:, m, g : g + 1],
                        )
                    k += 1

            # ---------------- scores^2 and patch argmax ----------------
            # scol = a2c - rowmax; E-part rowmax = KG + ln(sum)/SM
            mvf = constp.tile([PT, MT], F32, name="mvf")
            nc.vector.tensor_reduce(out=mvf, in_=mvs, axis=Axis.X, op=Alu.max)
            esf = constp.tile([PT, MT], F32, name="esf")
            nc.vector.tensor_reduce(out=esf, in_=esums, axis=Axis.X, op=Alu.add)
            lnv = constp.tile([PT, MT], F32, name="lnv")
            nc.scalar.activation(out=lnv, in_=esf, func=Act.Ln)
            sD = constp.tile([PT, MT], F32, name="sD")
            nc.vector.tensor_sub(sD, a2cs[0:PT, 0:MT], mvf)
            sE = constp.tile([PT, MT], F32, name="sE")
            nc.vector.scalar_tensor_tensor(
                out=sE, in0=lnv, scalar=-1.0 / SM, in1=a2K,
                op0=Alu.mult, op1=Alu.add,
            )
            nc.vector.tensor_tensor(
                out=scol[0:PT, 0:MT], in0=sD, in1=sE, op=Alu.min
            )

            # patch argmax: per-partition best patch via reduce + eq-mask
            # (indirect_copy indices are group-shared, so per-partition
            # gathers use select-by-equality instead)
            v1c = constp.tile([128, 1], F32, name="v1c")
            nc.vector.tensor_reduce(out=v1c, in_=scol, axis=Axis.X, op=Alu.max)
            am = constp.tile([128, 8], F32, name="am")
            a2sel = constp.tile([128, 1], F32, name="a2sel")
            nc.vector.scalar_tensor_tensor(
                out=am, in0=scol, scalar=v1c, in1=a2cs,
                op0=Alu.is_equal, op1=Alu.mult, accum_out=a2sel,
            )
            mm = constp.tile([128, 8], F32, name="mm")
            msel = constp.tile([128, 1], F32, name="msel")
            nc.vector.scalar_tensor_tensor(
                out=mm, in0=scol, scalar=v1c, in1=iotam[:, 0:8],
                op0=Alu.is_equal, op1=Alu.mult, accum_out=msel,
            )
            patchf = constp.tile([128, 1], F32, name="patchf")
            nc.vector.tensor_scalar_mul(patchf, msel, float(PT))
            nc.vector.tensor_add(patchf, patchf, pidxf)
            combo = constp.tile([128, 3], F32, name="combo")
            nc.vector.tensor_copy(combo[:, 0:1], v1c)
            nc.vector.tensor_copy(combo[:, 1:2], patchf)
            nc.vector.tensor_copy(combo[:, 2:3], a2sel)
            psT = psump.tile([128, NJ, 512], F32, name="psT", tag="ps")
            nc.tensor.transpose(psT[0:1, 0, 0:128], combo[:, 0:1], ident)
            nc.tensor.transpose(psT[0:1, 0, 128:256], combo[:, 1:2], ident)
            nc.tensor.transpose(psT[0:1, 0, 256:384], combo[:, 2:3], ident)
            mval = constp.tile([1, 8], F32, name="mval")
            midx = constp.tile([1, 8], U32, name="midx")
            nc.vector.max(out=mval, in_=psT[0:1, 0, 0:128])
            nc.vector.max_index(midx, mval, psT[0:1, 0, 0:128])
            rowp = psT[0:1, 0, 128:256]
            rowa = psT[0:1, 0, 256:384]
            pstar = nc.values_load(
                midx[0:1, 0:1], engines=[Eng.DVE],
                min_val=0, max_val=127, skip_runtime_bounds_check=True,
            )
            mpf = constp.tile([1, 1], F32, name="mpf")
            nc.vector.tensor_copy(mpf, rowp[0:1, ds(pstar, 1)])
            a2mp = constp.tile([1, 1], F32, name="a2mp")
            nc.vector.tensor_copy(a2mp, rowa[0:1, ds(pstar, 1)])
            mpu = constp.tile([1, 1], U32, name="mpu")
            nc.vector.tensor_copy(mpu, mpf)
            mp = nc.values_load(
                mpu, engines=[Eng.DVE],
                min_val=0, max_val=P - 1, skip_runtime_bounds_check=True,
            )
            ecol8a = constp.tile([128, 2, 1], FP8, name="ecol8a")
            nc.vector.tensor_copy(ecol8a, emt8a[:, :, ds(mp, 1)])
            ecol8b = constp.tile([128, 2, 1], FP8, name="ecol8b")
            nc.vector.tensor_copy(ecol8b, emt8b[:, :, ds(mp, 1)])
            if stage <= 1:
                nc.sync.dma_start(out=out_d[:], in_=mval[0:1, 0:1])
                return nc

            # ---------------- part A: v-row grid of the max patch ----------
            psG = psump.tile([128, NJ, 512], F32, name="psG", tag="ps")
            for c in range(128):
                nc.tensor.matmul(
                    psG[:, 0, c : c + 1],
                    lhsT=ct8a[:, :, c * 128 : (c + 1) * 128],
                    rhs=ecol8a, start=True, stop=False, perf_mode=PM.DoubleRow,
                )
                nc.tensor.matmul(
                    psG[:, 0, c : c + 1],
                    lhsT=ct8b[:, :, c * 128 : (c + 1) * 128],
                    rhs=ecol8b, start=False, stop=True, perf_mode=PM.DoubleRow,
                )
            vAc = constp.tile([128, 1], F32, name="vAc")
            nc.vector.tensor_reduce(
                out=vAc, in_=psG[:, 0, 0:128], axis=Axis.X, op=Alu.max
            )
            fm = constp.tile([128, 128], F32, name="fm")
            fA = constp.tile([128, 1], F32, name="fA")
            nc.vector.scalar_tensor_tensor(
                out=fm, in0=psG[:, 0, 0:128], scalar=vAc, in1=iotan,
                op0=Alu.is_equal, op1=Alu.mult, accum_out=fA,
            )
            gidxf = constp.tile([128, 1], F32, name="gidxf")
            nc.vector.tensor_scalar_mul(gidxf, fA, 128.0)
            nc.vector.tensor_add(gidxf, gidxf, pidxf)
            # lgrid = ln(a2c[mp] - v) = ln(d^2): distances come later via
            # d = exp(0.5 * lgrid), avoiding any sqrt table load.
            nc.tensor.matmul(
                psG[:, 1, 0:1], lhsT=ones1, rhs=a2mp, start=True, stop=True
            )
            biascol = constp.tile([128, 1], F32, name="biascol")
            nc.scalar.activation(out=biascol, in_=psG[:, 1, 0:1], func=Act.Copy)
            lgrid = constp.tile([128, 128], F32, name="lgrid")
            nc.scalar.activation(
                out=lgrid, in_=psG[:, 0, 0:128], func=Act.Ln,
                bias=biascol, scale=-1.0,
            )
            dgridX = constp.tile([128, 128], F32, name="dgridX")
            nc.scalar.activation(out=dgridX, in_=lgrid, func=Act.Exp, scale=0.5)
            egrid = constp.tile([128, 128], F32, name="egrid")
            nc.scalar.activation(out=egrid, in_=dgridX, func=Act.Exp)

            combo2 = constp.tile([128, 2], F32, name="combo2")
            nc.vector.tensor_copy(combo2[:, 0:1], vAc)
            nc.vector.tensor_copy(combo2[:, 1:2], gidxf)
            psT2 = psump.tile([128, NJ, 512], F32, name="psT2", tag="ps")
            nc.tensor.transpose(psT2[0:1, 0, 0:128], combo2[:, 0:1], ident)
            nc.tensor.transpose(psT2[0:1, 0, 128:256], combo2[:, 1:2], ident)
            mval2 = constp.tile([1, 8], F32, name="mval2")
            midx2 = constp.tile([1, 8], U32, name="midx2")
            nc.vector.max(out=mval2, in_=psT2[0:1, 0, 0:128])
            nc.vector.max_index(midx2, mval2, psT2[0:1, 0, 0:128])
            p2star = nc.values_load(
                midx2[0:1, 0:1], engines=[Eng.DVE],
                min_val=0, max_val=127, skip_runtime_bounds_check=True,
            )
            nnf = constp.tile([1, 1], F32, name="nnf")
            nc.vector.tensor_copy(nnf, psT2[0:1, 0, 128:256][0:1, ds(p2star, 1)])
            nnu = constp.tile([1, 1], U32, name="nnu")
            nc.vector.tensor_copy(nnu, nnf)
            nn = nc.values_load(
                nnu, engines=[Eng.DVE],
                min_val=0, max_val=N - 1, skip_runtime_bounds_check=True,
            )
            # exact score from the part-A grid max; escore = exp(score).
            # Both are off the critical path (part B runs meanwhile).
            s2ex = constp.tile([1, 1], F32, name="s2ex")
            nc.vector.tensor_sub(s2ex, a2mp, mval2[0:1, 0:1])
            lsc = constp.tile([1, 1], F32, name="lsc")
            nc.scalar.activation(out=lsc, in_=s2ex, func=Act.Ln)
            score = constp.tile([1, 1], F32, name="score")
            nc.scalar.activation(out=score, in_=lsc, func=Act.Exp, scale=0.5)
            escore = constp.tile([1, 1], F32, name="escore")
            nc.scalar.activation(out=escore, in_=score, func=Act.Exp)
            nscore = constp.tile([1, 1], F32, name="nscore")
            nc.vector.tensor_scalar_mul(nscore, score, -1.0)
            if stage <= 2:
                nc.sync.dma_start(out=out_d[:], in_=nnf)
                return nc

            # ---------------- part B: d_nn grid + top-9 support ----------
            # Support selection needs no sort/merge at all: kth_largest gives
            # the global 9th-largest of the whole v2 grid in one Pool op; a
            # >=threshold mask over the exp-distance grid then sums exp(d_sup)
            # for exactly the 9 support points.
            ccol8a = constp.tile([128, 2, 1], FP8, name="ccol8a")
            nc.vector.tensor_scalar_mul(ccol8a, ct8a[:, :, ds(nn, 1)], 2.0)
            nc.vector.tensor_scalar_mul(
                ccol8b[:, 0, :], ct8b[:, 0, ds(nn, 1)], 2.0
            )
            psH = psump.tile([128, NJ, 512], F32, name="psH", tag="ps")
            for c in range(128):
                nc.tensor.matmul(
                    psH[:, 0, c : c + 1],
                    lhsT=ct8a[:, :, c * 128 : (c + 1) * 128],
                    rhs=ccol8a, start=True, stop=False, perf_mode=PM.DoubleRow,
                )
                nc.tensor.matmul(
                    psH[:, 0, c : c + 1],
                    lhsT=ct8b[:, :, c * 128 : (c + 1) * 128],
                    rhs=ccol8b, start=False, stop=True, perf_mode=PM.DoubleRow,
                )
            gH = constp.tile([128, 128], F32, name="gH")
            nc.scalar.activation(out=gH, in_=psH[:, 0, 0:128], func=Act.Copy)
            # exact global 9th-largest: omq chosen so k_adj = 8 with a small
            # lerp toward desc[9]; any grid value >= lerped is in the top 9.
            kout = constp.tile([128, 2], F32, name="kout")
            nc.gpsimd.kth_largest(
                kout, gH, n_per_lane=128, k=16,
                quantile=1.0 - 2100000.0 / 4294967296.0,
            )
            psX = psump.tile([128, NJ, 512], F32, name="psX", tag="ps")
            nc.tensor.matmul(
                psX[:, 0, 0:1], lhsT=ones1, rhs=kout[0:1, 0:1],
                start=True, stop=True,
            )
            if stage <= 3:
                nc.sync.dma_start(out=out_d[:], in_=kout[0:1, 0:1])
                return nc

            # ---------------- softmax weight ----------------
            # one fused op: esel = (gH >= thr) * egrid, rowsum = sum(esel)
            esel = constp.tile([128, 128], F32, name="esel")
            rowsum = constp.tile([128, 1], F32, name="rowsum")
            nc.vector.scalar_tensor_tensor(
                out=esel, in0=gH, scalar=psX[:, 0, 0:1], in1=egrid,
                op0=Alu.is_ge, op1=Alu.mult, accum_out=rowsum,
            )
            nc.tensor.matmul(
                psX[0:1, 0, 2:3], lhsT=rowsum, rhs=ones128,
                start=True, stop=True,
            )
            rs = constp.tile([1, 1], F32, name="rs")
            nc.vector.reciprocal(rs, psX[0:1, 0, 2:3])
            p0 = constp.tile([1, 1], F32, name="p0")
            nc.vector.tensor_mul(p0, escore, rs)
            outv = constp.tile([1, 1], F32, name="outv")
            nc.vector.tensor_scalar(
                outv, p0, nscore, score, op0=Alu.mult, op1=Alu.add
            )
            nc.sync.dma_start(out=out_d[:], in_=outv)

    return nc


_NC = None


def _get_nc():
    global _NC
    if _NC is None:
        import os

        _NC = _build(stage=int(os.environ.get("KSTAGE", "99")))
    return _NC


def _prep_inputs(embedding, embedding_coreset):
    E = np.ascontiguousarray(np.asarray(embedding, dtype=np.float32))
    C = np.ascontiguousarray(np.asarray(embedding_coreset, dtype=np.float32))
    b2 = np.sum(C.astype(np.float64) * C, axis=1).astype(np.float32)
    b2c = b2 - C0
    nb2a = (-b2c).astype(FP8NP).astype(np.float32)
    nb2b = (-b2c - nb2a).astype(FP8NP).astype(np.float32)
    CT = C.T                                        # [D, N]
    ct8a = np.ascontiguousarray(
        np.stack([CT[0:128], CT[128:256]], axis=1).astype(FP8NP)
    ).reshape(128, 2 * N)
    bias_plane = np.zeros((128, N), np.float32)
    bias_plane[0] = nb2a
    bias_plane[1] = nb2b
    ct8b = np.ascontiguousarray(
        np.stack([CT[256:384], bias_plane], axis=1).astype(FP8NP)
    ).reshape(128, 2 * N)
    ident = np.eye(128, dtype=np.float32)
    ones_plane = np.zeros((128, P), np.float32)
    ones_plane[0] = 1.0
    ones_plane[1] = 1.0
    in_maps = []
    for i in range(B):
        Eb = E[i * P : (i + 1) * P]
        ET = (2.0 * Eb).T                           # [D, P]
        emt8a = np.ascontiguousarray(
            np.stack([ET[0:128], ET[128:256]], axis=1).astype(FP8NP)
        ).reshape(128, 2 * P)
        emt8b = np.ascontiguousarray(
            np.stack([ET[256:384], ones_plane], axis=1).astype(FP8NP)
        ).reshape(128, 2 * P)
        in_maps.append(
            {
                "ct8a": ct8a,
                "ct8b": ct8b,
                "emt8a": emt8a,
                "emt8b": emt8b,
                "er": np.ascontiguousarray(Eb),
                "ident": ident,
            }
        )
    return in_maps


def _run(embedding, embedding_coreset, batch_size, trace=False, **trace_kwargs):
    assert int(batch_size) == B
    in_maps = _prep_inputs(embedding, embedding_coreset)
    nc = _get_nc()
    res = run_bass_kernel_spmd(
        nc, in_maps, core_ids=list(range(B)), trace=trace, **trace_kwargs
    )
    out = np.array(
        [np.asarray(res.results[i]["out"]).reshape(-1)[0] for i in range(B)],
        dtype=np.float32,
    )
    return out, res


def kernel(embedding, embedding_coreset, batch_size):
    out, _ = _run(embedding, embedding_coreset, batch_size, trace=False)
    return out


# revision 39
# speedup vs baseline: 1.0025x; 1.0025x over previous
"""AnomalyScores (PatchCore-style 1-NN retrieval) Trainium2 kernel.

Sharding: data-parallel over the batch dim - core i owns batch i's 784
patches; the 16384x384 coreset is replicated on every core. All compute
is core-local; no collectives.

Per-core pipeline (fp8e4m3 DoubleRow matmuls, sign-flipped so every
selection is an argMAX):
  1. PE computes v = 2E @ C^T - b2' as [112 x 1024] PSUM tiles via two
     K=256 DoubleRow matmuls per 512-col bank; the centered-coreset-norm
     bias (-b2', two fp8 residual rows) rides in the second DoubleRow's
     spare contraction rows, so PSUM needs no preload.
  2. Drain (PSUM is readable only by DVE/ACT, one PSUM operand/instr):
     D-tiles: DVE tensor_reduce max -> mvs.  E-tiles: ACT computes
     exp(s*(v-K)) with accum_out -> per-tile sum; log-sum-exp recovers
     the row max to ~0.03 (only the patch argmax consumes these, so the
     softmax bias is harmless).  Tiles interleave D/E to balance both
     engines; Pool cannot touch tensors on HW and stays idle.
  3. scores^2 = a2c - rowmax, batched [112, 7]; patch argmax via PE
     transpose of a [128, 3] (score, patch-idx, a2c) combo. No DRAM.
  4. Part A: v-row of the max patch as a [128, 128] PSUM grid (256 tiny
     DoubleRow matmuls, rhs = dynamic fp8 column of emt); argmax -> nn.
     ACT takes ln(a2c[mp] - v) of the whole grid once (lgrid); sqrt is
     never needed: d = exp(0.5*ln(d^2)), keeping every activation in
     the one exp+ln+square table set (no tail table switches).
  5. Part B: same grid shape for 2*C[nn] vs C; top-4 per partition + PE
     transposes -> [1, 512] candidate row; max8/match_replace/max8 ->
     top-9 positions; indirect_copy gathers their lgrid values.
  6. softmax: d9 = exp(0.5*l9); exp+accum -> sum; score = min(d9);
     out = (1 - exp(score)/sum) * score. One scalar DMA per core.
"""

import sys

import numpy as np
import ml_dtypes

if "/opt/trn_rl_repo" not in sys.path:
    sys.path.insert(0, "/opt/trn_rl_repo")

import concourse.bass as bass
import concourse.mybir as mybir
import concourse.tile as tile
from concourse import bacc
from concourse.bass import ds
from concourse.bass_utils import run_bass_kernel_spmd

FP8NP = ml_dtypes.float8_e4m3
F32 = mybir.dt.float32
BF = mybir.dt.bfloat16
FP8 = mybir.dt.float8e4
U32 = mybir.dt.uint32
U16 = mybir.dt.uint16

B, P, D, N = 8, 784, 384, 16384
PT = 112          # patches per M-tile (7 * 112 = 784)
MT = 7
NG = 16           # N groups of 1024 (2 PSUM banks each)
NJ = 2
C0 = 384.0
BIG = 3.0e38
SM = 0.4          # log-sum-exp sharpness for the E-path drain
KG = 218.0        # global v-offset: |v_max - KG| stays well inside exp range
ND = 58           # of the 112 (m, g) tiles, this many go to the DVE path

Alu = mybir.AluOpType
Act = mybir.ActivationFunctionType
Axis = mybir.AxisListType
PM = mybir.MatmulPerfMode
Eng = mybir.EngineType


def _build(stage=99):
    nc = _build_inner(stage)
    nc.finalize()
    return nc


def _build_inner(stage=99):
    nc = bacc.Bacc("TRN2", target_bir_lowering=False, debug=False)

    ct8a_d = nc.dram_tensor("ct8a", [128, 2 * N], FP8, kind="ExternalInput")
    ct8b_d = nc.dram_tensor("ct8b", [128, 2 * N], FP8, kind="ExternalInput")
    emt8a_d = nc.dram_tensor("emt8a", [128, 2 * P], FP8, kind="ExternalInput")
    emt8b_d = nc.dram_tensor("emt8b", [128, 2 * P], FP8, kind="ExternalInput")
    er_d = nc.dram_tensor("er", [P, D], F32, kind="ExternalInput")
    id_d = nc.dram_tensor("ident", [128, 128], F32, kind="ExternalInput")
    out_d = nc.dram_tensor("out", [1], F32, kind="ExternalOutput")

    with tile.TileContext(nc) as tc:
        with (
            tc.tile_pool(name="constp", bufs=1) as constp,
            tc.tile_pool(name="workp", bufs=2) as workp,
            tc.tile_pool(name="psump", bufs=4, space="PSUM") as psump,
        ):
            # ---------------- resident inputs ----------------
            # DMA order matters: group 0 of the coreset + the emt operands
            # gate the first matmul; the identity matrix is tail-only.
            ct8a = constp.tile([128, 2, N], FP8, name="ct8a")
            ct8b = constp.tile([128, 2, N], FP8, name="ct8b")
            ct8a_v = ct8a_d[:, :].rearrange("p (i n) -> p i n", i=2)
            ct8b_v = ct8b_d[:, :].rearrange("p (i n) -> p i n", i=2)
            nc.sync.dma_start(out=ct8a[:, :, 0:1024], in_=ct8a_v[:, :, 0:1024])
            emt8a = constp.tile([128, 2, P], FP8, name="emt8a")
            nc.sync.dma_start(
                out=emt8a, in_=emt8a_d[:, :].rearrange("p (i n) -> p i n", i=2)
            )
            nc.sync.dma_start(out=ct8b[:, :, 0:1024], in_=ct8b_v[:, :, 0:1024])
            emt8b = constp.tile([128, 2, P], FP8, name="emt8b")
            nc.sync.dma_start(
                out=emt8b, in_=emt8b_d[:, :].rearrange("p (i n) -> p i n", i=2)
            )
            for g in range(1, NG):
                lo, hi = g * 1024, (g + 1) * 1024
                nc.sync.dma_start(out=ct8a[:, :, lo:hi], in_=ct8a_v[:, :, lo:hi])
                nc.sync.dma_start(out=ct8b[:, :, lo:hi], in_=ct8b_v[:, :, lo:hi])
            ident = constp.tile([128, 128], F32, name="ident")
            nc.sync.dma_start(out=ident, in_=id_d[:, :])

            # PE pstate warmup on a zeroed junk tile while DMAs land.
            wj = constp.tile([128, 512], BF, name="wj")
            nc.vector.memset(wj, 0.0)
            for _ in range(12):
                wps = psump.tile([128, NJ, 512], F32, name="wps", tag="ps")
                nc.tensor.matmul(
                    wps[:, 0, :], lhsT=wj[:, 0:128], rhs=wj, start=True, stop=True
                )

            pidx = constp.tile([128, 1], U32, name="pidx")
            nc.gpsimd.iota(pidx, pattern=[[0, 1]], base=0, channel_multiplier=1)
            ones1 = constp.tile([1, 128], F32, name="ones1")
            nc.vector.memset(ones1, 1.0)

            # prezeroed tiles touched by indirect gathers / partial writes
            a2cs = constp.tile([128, 8], F32, name="a2cs")
            nc.vector.memset(a2cs, 0.0)
            scol = constp.tile([128, 8], F32, name="scol")
            nc.vector.memset(scol, -BIG)
            # part-B rhs second k-pair: plane 1 is the constant bias rhs
            # (rows 0,1 = 1.0, rest 0), plane 0 filled at tail time.
            ccol8b = constp.tile([128, 2, 1], FP8, name="ccol8b")
            nc.vector.memset(ccol8b, 0.0)
            nc.vector.memset(ccol8b[0:2, 1, 0:1], 1.0)

            pidxf = constp.tile([128, 1], F32, name="pidxf")
            nc.vector.tensor_copy(pidxf, pidx)
            iotam = constp.tile([128, 9], F32, name="iotam")
            nc.gpsimd.iota(iotam, pattern=[[1, 9]], base=0, channel_multiplier=0, allow_small_or_imprecise_dtypes=True)
            iotan = constp.tile([128, 128], F32, name="iotan")
            nc.gpsimd.iota(iotan, pattern=[[1, 128]], base=0, channel_multiplier=0, allow_small_or_imprecise_dtypes=True)
            ones128 = constp.tile([128, 1], F32, name="ones128")
            nc.vector.memset(ones128, 1.0)
            ebias = constp.tile([PT, 1], F32, name="ebias")
            nc.vector.memset(ebias, -SM * KG)
            # drain accumulators: row maxes (D) and exp-sums (E) per (m, g)
            mvs = constp.tile([PT, MT, NG], F32, name="mvs")
            nc.vector.memset(mvs, -BIG)
            esums = constp.tile([PT, MT, NG], F32, name="esums")
            nc.vector.memset(esums, 0.0)

            # Preload the exp+ln+square act table once (set 6) while DMAs
            # stream; the greedy auto-inserter would otherwise ping-pong
            # between the exp-only and ln-only sets in the tail.
            nc.scalar.add_instruction(
                mybir.InstLoadActFuncSet(
                    name=nc.get_next_instruction_name(), ins=[], outs=[],
                    act_func_set_id=6,
                )
            )

            # ---------------- a2 + C0 per M-tile (ACT; overlaps main) -----
            for m in range(MT):
                er_sb = workp.tile([PT, D], F32, name="er_sb", tag="er_sb")
                nc.sync.dma_start(out=er_sb, in_=er_d[m * PT : (m + 1) * PT, :])
                sq = workp.tile([PT, D], F32, name="sq", tag="sq")
                a2r = workp.tile([PT, 1], F32, name="a2r", tag="a2r")
                nc.scalar.activation(out=sq, in_=er_sb, func=Act.Square, accum_out=a2r)
                nc.vector.tensor_scalar_add(a2cs[0:PT, m : m + 1], a2r, C0)

            a2K = constp.tile([PT, MT], F32, name="a2K")
            nc.vector.tensor_scalar_add(a2K, a2cs[0:PT, 0:MT], -KG)

            # ---------------- main distance pass ----------------
            k = 0
            for g in range(NG):
                for m in range(MT):
                    ps = psump.tile([128, NJ, 512], F32, name="ps", tag="ps")
                    for j in range(NJ):
                        col = (g * NJ + j) * 512
                        nc.tensor.matmul(
                            ps[0:PT, j, :],
                            lhsT=emt8a[:, :, m * PT : (m + 1) * PT],
                            rhs=ct8a[:, :, col : col + 512],
                            start=True,
                            stop=False,
                            perf_mode=PM.DoubleRow,
                        )
                        nc.tensor.matmul(
                            ps[0:PT, j, :],
                            lhsT=emt8b[:, :, m * PT : (m + 1) * PT],
                            rhs=ct8b[:, :, col : col + 512],
                            start=False,
                            stop=True,
                            perf_mode=PM.DoubleRow,
                        )
                    if (k * ND) % 112 < ND:
                        nc.vector.tensor_reduce(
                            out=mvs[:, m, g : g + 1], in_=ps[0:PT],
                            axis=Axis.XY, op=Alu.max,
                        )
                    else:
                        scrE = workp.tile(
                            [PT, NJ, 512], BF, name="scrE", tag="scrE", bufs=3
                        )
                        nc.scalar.activation(
                            out=scrE, in_=ps[0:PT], func=Act.Exp,
                            scale=SM, bias=ebias,
                            accum_out=esums[:, m, g : g + 1],
                        )
                    k += 1

            # ---------------- scores^2 and patch argmax ----------------
            # scol = a2c - rowmax; E-part rowmax = KG + ln(sum)/SM
            mvf = constp.tile([PT, MT], F32, name="mvf")
            nc.vector.tensor_reduce(out=mvf, in_=mvs, axis=Axis.X, op=Alu.max)
            esf = constp.tile([PT, MT], F32, name="esf")
            nc.vector.tensor_reduce(out=esf, in_=esums, axis=Axis.X, op=Alu.add)
            lnv = constp.tile([PT, MT], F32, name="lnv")
            nc.scalar.activation(out=lnv, in_=esf, func=Act.Ln)
            sD = constp.tile([PT, MT], F32, name="sD")
            nc.vector.tensor_sub(sD, a2cs[0:PT, 0:MT], mvf)
            sE = constp.tile([PT, MT], F32, name="sE")
            nc.vector.scalar_tensor_tensor(
                out=sE, in0=lnv, scalar=-1.0 / SM, in1=a2K,
                op0=Alu.mult, op1=Alu.add,
            )
            nc.vector.tensor_tensor(
                out=scol[0:PT, 0:MT], in0=sD, in1=sE, op=Alu.min
            )

            # patch argmax: per-partition best patch via reduce + eq-mask
            # (indirect_copy indices are group-shared, so per-partition
            # gathers use select-by-equality instead)
            v1c = constp.tile([128, 1], F32, name="v1c")
            nc.vector.tensor_reduce(out=v1c, in_=scol, axis=Axis.X, op=Alu.max)
            am = constp.tile([128, 8], F32, name="am")
            a2sel = constp.tile([128, 1], F32, name="a2sel")
            nc.vector.scalar_tensor_tensor(
                out=am, in0=scol, scalar=v1c, in1=a2cs,
                op0=Alu.is_equal, op1=Alu.mult, accum_out=a2sel,
            )
            mm = constp.tile([128, 8], F32, name="mm")
            msel = constp.tile([128, 1], F32, name="msel")
            nc.vector.scalar_tensor_tensor(
                out=mm, in0=scol, scalar=v1c, in1=iotam[:, 0:8],
                op0=Alu.is_equal, op1=Alu.mult, accum_out=msel,
            )
            patchf = constp.tile([128, 1], F32, name="patchf")
            nc.vector.tensor_scalar_mul(patchf, msel, float(PT))
            nc.vector.tensor_add(patchf, patchf, pidxf)
            combo = constp.tile([128, 3], F32, name="combo")
            nc.vector.tensor_copy(combo[:, 0:1], v1c)
            nc.vector.tensor_copy(combo[:, 1:2], patchf)
            nc.vector.tensor_copy(combo[:, 2:3], a2sel)
            psT = psump.tile([128, NJ, 512], F32, name="psT", tag="ps")
            nc.tensor.transpose(psT[0:1, 0, 0:128], combo[:, 0:1], ident)
            nc.tensor.transpose(psT[0:1, 0, 128:256], combo[:, 1:2], ident)
            nc.tensor.transpose(psT[0:1, 0, 256:384], combo[:, 2:3], ident)
            mval = constp.tile([1, 8], F32, name="mval")
            midx = constp.tile([1, 8], U32, name="midx")
            nc.vector.max(out=mval, in_=psT[0:1, 0, 0:128])
            nc.vector.max_index(midx, mval, psT[0:1, 0, 0:128])
            rowp = psT[0:1, 0, 128:256]
            rowa = psT[0:1, 0, 256:384]
            pstar = nc.values_load(
                midx[0:1, 0:1], engines=[Eng.DVE],
                min_val=0, max_val=127, skip_runtime_bounds_check=True,
            )
            mpf = constp.tile([1, 1], F32, name="mpf")
            nc.vector.tensor_copy(mpf, rowp[0:1, ds(pstar, 1)])
            a2mp = constp.tile([1, 1], F32, name="a2mp")
            nc.vector.tensor_copy(a2mp, rowa[0:1, ds(pstar, 1)])
            mpu = constp.tile([1, 1], U32, name="mpu")
            nc.vector.tensor_copy(mpu, mpf)
            mp = nc.values_load(
                mpu, engines=[Eng.DVE],
                min_val=0, max_val=P - 1, skip_runtime_bounds_check=True,
            )
            ecol8a = constp.tile([128, 2, 1], FP8, name="ecol8a")
            nc.vector.tensor_copy(ecol8a, emt8a[:, :, ds(mp, 1)])
            ecol8b = constp.tile([128, 2, 1], FP8, name="ecol8b")
            nc.vector.tensor_copy(ecol8b, emt8b[:, :, ds(mp, 1)])
            if stage <= 1:
                nc.sync.dma_start(out=out_d[:], in_=mval[0:1, 0:1])
                return nc

            # ---------------- part A: v-row grid of the max patch ----------
            psG = psump.tile([128, NJ, 512], F32, name="psG", tag="ps")
            for c in range(128):
                nc.tensor.matmul(
                    psG[:, 0, c : c + 1],
                    lhsT=ct8a[:, :, c * 128 : (c + 1) * 128],
                    rhs=ecol8a, start=True, stop=False, perf_mode=PM.DoubleRow,
                )
                nc.tensor.matmul(
                    psG[:, 0, c : c + 1],
                    lhsT=ct8b[:, :, c * 128 : (c + 1) * 128],
                    rhs=ecol8b, start=False, stop=True, perf_mode=PM.DoubleRow,
                )
            vAc = constp.tile([128, 1], F32, name="vAc")
            nc.vector.tensor_reduce(
                out=vAc, in_=psG[:, 0, 0:128], axis=Axis.X, op=Alu.max
            )
            fm = constp.tile([128, 128], F32, name="fm")
            fA = constp.tile([128, 1], F32, name="fA")
            nc.vector.scalar_tensor_tensor(
                out=fm, in0=psG[:, 0, 0:128], scalar=vAc, in1=iotan,
                op0=Alu.is_equal, op1=Alu.mult, accum_out=fA,
            )
            gidxf = constp.tile([128, 1], F32, name="gidxf")
            nc.vector.tensor_scalar_mul(gidxf, fA, 128.0)
            nc.vector.tensor_add(gidxf, gidxf, pidxf)
            # lgrid = ln(a2c[mp] - v) = ln(d^2): distances come later via
            # d = exp(0.5 * lgrid), avoiding any sqrt table load.
            nc.tensor.matmul(
                psG[:, 1, 0:1], lhsT=ones1, rhs=a2mp, start=True, stop=True
            )
            biascol = constp.tile([128, 1], F32, name="biascol")
            nc.scalar.activation(out=biascol, in_=psG[:, 1, 0:1], func=Act.Copy)
            lgrid = constp.tile([128, 128], F32, name="lgrid")
            nc.scalar.activation(
                out=lgrid, in_=psG[:, 0, 0:128], func=Act.Ln,
                bias=biascol, scale=-1.0,
            )
            dgridX = constp.tile([128, 128], F32, name="dgridX")
            nc.scalar.activation(out=dgridX, in_=lgrid, func=Act.Exp, scale=0.5)
            egrid = constp.tile([128, 128], F32, name="egrid")
            nc.scalar.activation(out=egrid, in_=dgridX, func=Act.Exp)

            combo2 = constp.tile([128, 2], F32, name="combo2")
            nc.vector.tensor_copy(combo2[:, 0:1], vAc)
            nc.vector.tensor_copy(combo2[:, 1:2], gidxf)
            psT2 = psump.tile([128, NJ, 512], F32, name="psT2", tag="ps")
            nc.tensor.transpose(psT2[0:1, 0, 0:128], combo2[:, 0:1], ident)
            nc.tensor.transpose(psT2[0:1, 0, 128:256], combo2[:, 1:2], ident)
            mval2 = constp.tile([1, 8], F32, name="mval2")
            midx2 = constp.tile([1, 8], U32, name="midx2")
            nc.vector.max(out=mval2, in_=psT2[0:1, 0, 0:128])
            nc.vector.max_index(midx2, mval2, psT2[0:1, 0, 0:128])
            p2star = nc.values_load(
                midx2[0:1, 0:1], engines=[Eng.DVE],
                min_val=0, max_val=127, skip_runtime_bounds_check=True,
            )
            nnf = constp.tile([1, 1], F32, name="nnf")
            nc.vector.tensor_copy(nnf, psT2[0:1, 0, 128:256][0:1, ds(p2star, 1)])
            nnu = constp.tile([1, 1], U32, name="nnu")
            nc.vector.tensor_copy(nnu, nnf)
            nn = nc.values_load(
                nnu, engines=[Eng.DVE],
                min_val=0, max_val=N - 1, skip_runtime_bounds_check=True,
            )
            # exact score from the part-A grid max; escore = exp(score).
            # Both are off the critical path (part B runs meanwhile).
            s2ex = constp.tile([1, 1], F32, name="s2ex")
            nc.vector.tensor_sub(s2ex, a2mp, mval2[0:1, 0:1])
            lsc = constp.tile([1, 1], F32, name="lsc")
            nc.scalar.activation(out=lsc, in_=s2ex, func=Act.Ln)
            score = constp.tile([1, 1], F32, name="score")
            nc.scalar.activation(out=score, in_=lsc, func=Act.Exp, scale=0.5)
            escore = constp.tile([1, 1], F32, name="escore")
            nc.scalar.activation(out=escore, in_=score, func=Act.Exp)
            nscore = constp.tile([1, 1], F32, name="nscore")
            nc.vector.tensor_scalar_mul(nscore, score, -1.0)
            if stage <= 2:
                nc.sync.dma_start(out=out_d[:], in_=nnf)
                return nc

            # ---------------- part B: d_nn grid + top-9 support ----------
            # Support selection needs no sort/merge at all: kth_largest gives
            # the global 9th-largest of the whole v2 grid in one Pool op; a
            # >=threshold mask over the exp-distance grid then sums exp(d_sup)
            # for exactly the 9 support points.
            ccol8a = constp.tile([128, 2, 1], FP8, name="ccol8a")
            nc.vector.tensor_scalar_mul(ccol8a, ct8a[:, :, ds(nn, 1)], 2.0)
            nc.vector.tensor_scalar_mul(
                ccol8b[:, 0, :], ct8b[:, 0, ds(nn, 1)], 2.0
            )
            psH = psump.tile([128, NJ, 512], F32, name="psH", tag="ps")
            for c in range(128):
                nc.tensor.matmul(
                    psH[:, 0, c : c + 1],
                    lhsT=ct8a[:, :, c * 128 : (c + 1) * 128],
                    rhs=ccol8a, start=True, stop=False, perf_mode=PM.DoubleRow,
                )
                nc.tensor.matmul(
                    psH[:, 0, c : c + 1],
                    lhsT=ct8b[:, :, c * 128 : (c + 1) * 128],
                    rhs=ccol8b, start=False, stop=True, perf_mode=PM.DoubleRow,
                )
            gH = constp.tile([128, 128], F32, name="gH")
            nc.scalar.activation(out=gH, in_=psH[:, 0, 0:128], func=Act.Copy)
            # exact global 9th-largest: omq chosen so k_adj = 8 with a small
            # lerp toward desc[9]; any grid value >= lerped is in the top 9.
            kout = constp.tile([128, 2], F32, name="kout")
            nc.gpsimd.kth_largest(
                kout, gH, n_per_lane=128, k=16,
                quantile=1.0 - 2100000.0 / 4294967296.0,
            )
            psX = psump.tile([128, NJ, 512], F32, name="psX", tag="ps")
            nc.tensor.matmul(
                psX[:, 0, 0:1], lhsT=ones1, rhs=kout[0:1, 0:1],
                start=True, stop=True,
            )
            if stage <= 3:
                nc.sync.dma_start(out=out_d[:], in_=kout[0:1, 0:1])
                return nc

            # ---------------- softmax weight ----------------
            # one fused op: esel = (gH >= thr) * egrid, rowsum = sum(esel)
            esel = constp.tile([128, 128], F32, name="esel")
            rowsum = constp.tile([128, 1], F32, name="rowsum")
            nc.vector.scalar_tensor_tensor(
                out=esel, in0=gH, scalar=psX[:, 0, 0:1], in1=egrid,
                op0=Alu.is_ge, op1=Alu.mult, accum_out=rowsum,
            )
            nc.tensor.matmul(
                psX[0:1, 0, 2:3], lhsT=rowsum, rhs=ones128,
                start=True, stop=True,
            )
            rs = constp.tile([1, 1], F32, name="rs")
            nc.vector.reciprocal(rs, psX[0:1, 0, 2:3])
            p0 = constp.tile([1, 1], F32, name="p0")
            nc.vector.tensor_mul(p0, escore, rs)
            outv = constp.tile([1, 1], F32, name="outv")
            nc.vector.tensor_scalar(
                outv, p0, nscore, score, op0=Alu.mult, op1=Alu.add
            )
            nc.sync.dma_start(out=out_d[:], in_=outv)

    return nc


_NC = None


def _get_nc():
    global _NC
    if _NC is None:
        import os

        _NC = _build(stage=int(os.environ.get("KSTAGE", "99")))
    return _NC


def _prep_inputs(embedding, embedding_coreset):
    E = np.ascontiguousarray(np.asarray(embedding, dtype=np.float32))
    C = np.ascontiguousarray(np.asarray(embedding_coreset, dtype=np.float32))
    b2 = np.sum(C.astype(np.float64) * C, axis=1).astype(np.float32)
    b2c = b2 - C0
    nb2a = (-b2c).astype(FP8NP).astype(np.float32)
    nb2b = (-b2c - nb2a).astype(FP8NP).astype(np.float32)
    CT = C.T                                        # [D, N]
    ct8a = np.ascontiguousarray(
        np.stack([CT[0:128], CT[128:256]], axis=1).astype(FP8NP)
    ).reshape(128, 2 * N)
    bias_plane = np.zeros((128, N), np.float32)
    bias_plane[0] = nb2a
    bias_plane[1] = nb2b
    ct8b = np.ascontiguousarray(
        np.stack([CT[256:384], bias_plane], axis=1).astype(FP8NP)
    ).reshape(128, 2 * N)
    ident = np.eye(128, dtype=np.float32)
    ones_plane = np.zeros((128, P), np.float32)
    ones_plane[0] = 1.0
    ones_plane[1] = 1.0
    in_maps = []
    for i in range(B):
        Eb = E[i * P : (i + 1) * P]
        ET = (2.0 * Eb).T                           # [D, P]
        emt8a = np.ascontiguousarray(
            np.stack([ET[0:128], ET[128:256]], axis=1).astype(FP8NP)
        ).reshape(128, 2 * P)
        emt8b = np.ascontiguousarray(
            np.stack([ET[256:384], ones_plane], axis=1).astype(FP8NP)
        ).reshape(128, 2 * P)
        in_maps.append(
            {
                "ct8a": ct8a,
                "ct8b": ct8b,
                "emt8a": emt8a,
                "emt8b": emt8b,
                "er": np.ascontiguousarray(Eb),
                "ident": ident,
            }
        )
    return in_maps


def _run(embedding, embedding_coreset, batch_size, trace=False, **trace_kwargs):
    assert int(batch_size) == B
    in_maps = _prep_inputs(embedding, embedding_coreset)
    nc = _get_nc()
    res = run_bass_kernel_spmd(
        nc, in_maps, core_ids=list(range(B)), trace=trace, **trace_kwargs
    )
    out = np.array(
        [np.asarray(res.results[i]["out"]).reshape(-1)[0] for i in range(B)],
        dtype=np.float32,
    )
    return out, res


def kernel(embedding, embedding_coreset, batch_size):
    out, _ = _run(embedding, embedding_coreset, batch_size, trace=False)
    return out
